# revision 4
# baseline (speedup 1.0000x reference)
"""Trainium2 Bass kernel for nn_GATWithMultipleLinearLayers (2-layer GAT +
residual linears + 3-layer MLP + classifier + softmax), sharded over 8
NeuronCores.

Strategy (graph/data parallel, dst-owner sharding):
  - Nodes padded to 50176 = 8 * 6272 rows; core c owns rows [c*6272,(c+1)*6272).
  - Edges grouped by dst owner, dst window (128 nodes), and src bank (2 banks
    per owner block so gather tables stay under the int16 index limit of the
    custom dma_gather instruction). Edge slots padded to a static tile count.
  - conv1: every core computes the full gather table T1 = [x@W1 | a_s1]
    (replicated dense compute, avoids an AllGather of h1). Edge softmax +
    scatter are done per 128-dst-node window with a one-hot matmul:
        out[d,:] = sum_e (dst_e==d) * p_e * h[src_e],  s[d] = sum_e (..) * p_e
    where p_e = exp(leaky_relu(a_s[src_e] + a_d[dst_e])) and division by s
    happens once per window (mathematically identical to the reference's
    stable edge softmax; logits are small so exp cannot overflow).
  - conv2: each core computes its own rows of the T2 table = [h1'@W2 | a_s2],
    which is AllGathered (2 collectives), then the same edge pipeline runs.
  - MLP/classifier/softmax run on owned rows only, channels-on-partitions to
    avoid transposes.
  All matmul inputs are fp16 (validated ~2.6e-4 abs err vs fp32 reference);
  accumulation is fp32 in PSUM; vector math is fp32.
"""
import sys
import numpy as np

sys.path.insert(0, "/opt/trn_rl_repo")
sys.path.insert(0, "/opt/trn_rl_repo/concourse")

import concourse.bass as bass
import concourse.bacc as bacc
import concourse.mybir as mybir
from concourse import tile
from concourse.bass_utils import run_bass_kernel_spmd

F16 = mybir.dt.float16
F32 = mybir.dt.float32
I16 = mybir.dt.int16
AOP = mybir.AluOpType
ACT = mybir.ActivationFunctionType

# ---- problem constants (hardcoded; shapes fixed by the problem spec) ----
N = 50000
E = 800000
D_IN = 256
H1 = 512
H2 = 256
NCLS = 5
NEG = 0.2
M = 8                      # cores
OWN = 6272                 # 49*128 rows per owner block
NW = 49                    # dst windows per core
NPAD = M * OWN             # 50176
NT_ALL = NPAD // 128       # 392 global row tiles
# src bank split within each owner block (window-aligned, tables < 32768 rows)
QSTART = (0, 3200)
QLEN = (3200, 3072)
BANK_ROWS = (M * QLEN[0], M * QLEN[1])          # 25600, 24576
# static tiles per window per bank (max over the seed-0 graph is 9/9;
# rebuilt automatically if a different graph needs more)
T_BANK_DEFAULT = (9, 9)
W_T1 = 640                 # fp16 table row: [h1(512) | a_s1 | pad] (1280B %256)
W_T2 = 384                 # fp16 table row: [h2(256) | a_s2 | pad] (768B %256)

_CACHE = {}


import os
PHASES = int(os.environ.get("KPHASES", "7"))
ESUB = int(os.environ.get("KESUB", "99"))
KDUMP = os.environ.get("KDUMP", "0") == "1"


def _build_program(t_bank):
    T0, T1T = t_bank
    T = T0 + T1T
    S0, S1 = T0 * 8, T1T * 8          # int16 idx cols per bank (num_idxs/16)
    S = S0 + S1

    nc = bacc.Bacc("TRN2", target_bir_lowering=False, debug=False, num_devices=M)

    # ---------------- inputs ----------------
    def din(name, shape, dt):
        return nc.dram_tensor(name, list(shape), dt, kind="ExternalInput")

    xTg = din("xTg", (D_IN, NPAD), F16)       # x.T, padded (replicated)
    xTo = din("xTo", (D_IN, OWN), F16)        # own slice of x.T (per core)
    w1s = din("w1s", (128, 2 * H1), F16)      # W1 k-tiles side by side
    a1s = din("a1s", (128, 2 * H1), F16)
    w2s = din("w2s", (128, 4 * H2), F16)
    a2s = din("a2s", (128, 4 * H2), F16)
    hws = din("hws", (128, 6 * H2), F16)      # Hw1..3, k-tiles
    fcws = din("fcws", (128, 2 * NCLS), F16)
    wv1 = din("wv1", (128, 4), F16)           # [was1_k0 was1_k1 wad1_k0 wad1_k1]
    wv2 = din("wv2", (128, 8), F16)           # [was2_k0..3 wad2_k0..3]
    b1r = din("b1r", (128, H1), F32)          # b_conv1+b1 replicated rows
    b2r = din("b2r", (128, H2), F32)
    hbs = din("hbs", (128, 6), F32)           # Hb[l][ch*128+p] at col l*2+ch
    fcbr = din("fcbr", (128, NCLS), F32)
    iota16 = din("iota16", (128, 128), F16)
    idn16 = din("idn16", (128, 128), F16)
    midx = din("midx", (NW, 128, S), I16)     # wrapped gather indices
    mdr = din("mdr", (NW, 128, T), F32)       # dst_rel per edge slot (-1 pad)

    outp = nc.dram_tensor("outp", [OWN, NCLS], F32, kind="ExternalOutput")

    # ---------------- internal DRAM ----------------
    ik = dict(kind="ExternalOutput") if KDUMP else {}
    t1a = nc.dram_tensor("t1a", [BANK_ROWS[0], W_T1], F16, **ik)
    t1b = nc.dram_tensor("t1b", [BANK_ROWS[1], W_T1], F16, **ik)
    res1 = nc.dram_tensor("res1", [OWN, H1], F32, **ik)
    h1T = nc.dram_tensor("h1T", [H1, OWN], F16, **ik)
    t2inA = nc.dram_tensor("t2inA", [QLEN[0], W_T2], F16)
    t2inB = nc.dram_tensor("t2inB", [QLEN[1], W_T2], F16)
    t2a = nc.dram_tensor("t2a", [M, QLEN[0], W_T2], F16, addr_space="Shared")
    t2b = nc.dram_tensor("t2b", [M, QLEN[1], W_T2], F16, addr_space="Shared")
    res2 = nc.dram_tensor("res2", [OWN, H2], F32, **ik)
    if KDUMP:
        lgdbg = nc.dram_tensor("lgdbg", [OWN, NCLS], F32, kind="ExternalOutput")
        t2indbg = nc.dram_tensor("t2indbg", [128, W_T2], F16, kind="ExternalOutput")
        t2adbg = nc.dram_tensor("t2adbg", [3 * 128, W_T2], F16, kind="ExternalOutput")
    h2T = nc.dram_tensor("h2T", [H2, OWN], F16, **ik)

    rg = [list(range(M))]

    with tile.TileContext(nc) as tc:
        with tc.tile_pool(name="const", bufs=1) as cst, \
             tc.tile_pool(name="xin", bufs=3) as xin, \
             tc.tile_pool(name="hx", bufs=3) as hxp, \
             tc.tile_pool(name="gth", bufs=2) as gth, \
             tc.tile_pool(name="ohb", bufs=2) as ohb, \
             tc.tile_pool(name="ew", bufs=3) as ew, \
             tc.tile_pool(name="epi", bufs=2) as epi, \
             tc.tile_pool(name="psA", bufs=2, space="PSUM") as psA, \
             tc.tile_pool(name="psS", bufs=2, space="PSUM") as psS, \
             tc.tile_pool(name="psM", bufs=3, space="PSUM") as psM:

            # ---- persistent constants in SBUF ----
            w1_t = cst.tile([128, 2 * H1], F16)
            a1_t = cst.tile([128, 2 * H1], F16)
            w2_t = cst.tile([128, 4 * H2], F16)
            a2_t = cst.tile([128, 4 * H2], F16)
            hw_t = cst.tile([128, 6 * H2], F16)
            fcw_t = cst.tile([128, 2 * NCLS], F16)
            wv1_t = cst.tile([128, 4], F16)
            wv2_t = cst.tile([128, 8], F16)
            b1r_t = cst.tile([128, H1], F32)
            b2r_t = cst.tile([128, H2], F32)
            hbs_t = cst.tile([128, 6], F32)
            fcbr_t = cst.tile([128, NCLS], F32)
            iota_t = cst.tile([128, 128], F16)
            idn_t = cst.tile([128, 128], F16)
            ones_t = cst.tile([128, 1], F16)
            ad1_t = cst.tile([128, NW], F16)
            ad2_t = cst.tile([128, NW], F16)
            for t_, src_ in ((w1_t, w1s), (a1_t, a1s), (w2_t, w2s), (a2_t, a2s),
                             (hw_t, hws), (fcw_t, fcws), (wv1_t, wv1),
                             (wv2_t, wv2), (b1r_t, b1r), (b2r_t, b2r),
                             (hbs_t, hbs), (fcbr_t, fcbr), (iota_t, iota16),
                             (idn_t, idn16)):
                nc.sync.dma_start(out=t_[:], in_=src_[:, :])
            nc.vector.memset(ones_t[:], 1.0)

            # =========== D1a: T1 = [x@W1 | a_s1] for ALL rows ===========
            CH = 4  # m-tiles per x chunk load
            for i0 in range(0, NT_ALL, CH):
                nch = min(CH, NT_ALL - i0)
                xc0 = xin.tile([128, CH * 128], F16, tag="xc0")
                xc1 = xin.tile([128, CH * 128], F16, tag="xc1")
                nc.sync.dma_start(out=xc0[:, 0:nch * 128],
                                  in_=xTg[0:128, i0 * 128:(i0 + nch) * 128])
                nc.sync.dma_start(out=xc1[:, 0:nch * 128],
                                  in_=xTg[128:256, i0 * 128:(i0 + nch) * 128])
                for j in range(nch):
                    i = i0 + j
                    xt = (xc0[:, j * 128:(j + 1) * 128],
                          xc1[:, j * 128:(j + 1) * 128])
                    h_ps = psA.tile([128, H1], F32, space="PSUM", tag="acc")
                    as_ps = psS.tile([128, 8], F32, space="PSUM", tag="s")
                    for k in range(2):
                        nc.tensor.matmul(out=h_ps[:], lhsT=xt[k],
                                         rhs=w1_t[:, k * H1:(k + 1) * H1],
                                         start=(k == 0), stop=(k == 1))
                        nc.tensor.matmul(out=as_ps[:, 0:1], lhsT=xt[k],
                                         rhs=wv1_t[:, k:k + 1],
                                         start=(k == 0), stop=(k == 1))
                    hx = hxp.tile([128, W_T1], F16, tag="hx")
                    nc.scalar.activation(out=hx[:, 0:H1], in_=h_ps[:], func=ACT.Copy)
                    nc.vector.tensor_copy(out=hx[:, H1:H1 + 1], in_=as_ps[:, 0:1])
                    o, w = i // NW, i % NW
                    if w < 25:
                        dst = t1a[o * QLEN[0] + w * 128:o * QLEN[0] + (w + 1) * 128, :]
                    else:
                        dst = t1b[o * QLEN[1] + (w - 25) * 128:
                                  o * QLEN[1] + (w - 24) * 128, :]
                    nc.sync.dma_start(out=dst, in_=hx[:])

            # =========== D1b: own residual + a_d1 ===========
            for w in range(NW if PHASES >= 2 else 0):
                xo0 = xin.tile([128, 128], F16, tag="xo0")
                xo1 = xin.tile([128, 128], F16, tag="xo1")
                nc.sync.dma_start(out=xo0[:], in_=xTo[0:128, w * 128:(w + 1) * 128])
                nc.sync.dma_start(out=xo1[:], in_=xTo[128:256, w * 128:(w + 1) * 128])
                ad_ps = psS.tile([128, 8], F32, space="PSUM", tag="s")
                r_ps = psA.tile([128, H1], F32, space="PSUM", tag="acc")
                for k, xo in enumerate((xo0, xo1)):
                    nc.tensor.matmul(out=ad_ps[:, 0:1], lhsT=xo[:],
                                     rhs=wv1_t[:, 2 + k:3 + k],
                                     start=(k == 0), stop=(k == 1))
                    nc.tensor.matmul(out=r_ps[:], lhsT=xo[:],
                                     rhs=a1_t[:, k * H1:(k + 1) * H1],
                                     start=(k == 0), stop=(k == 1))
                nc.vector.tensor_copy(out=ad1_t[:, w:w + 1], in_=ad_ps[:, 0:1])
                r_sb = epi.tile([128, H1], F32, tag="r1")
                nc.vector.tensor_tensor(out=r_sb[:], in0=r_ps[:], in1=b1r_t[:],
                                        op=AOP.add)
                nc.sync.dma_start(out=res1[w * 128:(w + 1) * 128, :], in_=r_sb[:])

            # =========== edge phase helper ===========
            def edge_phase(w, H, WT, tabs, adcols, res_dram, do_relu, outT_dram):
                idx_t = ew.tile([128, S], I16, tag="idx")
                nc.sync.dma_start(out=idx_t[:], in_=midx[w, :, :])
                dr_t = ew.tile([128, T], F32, tag="dr")
                nc.sync.dma_start(out=dr_t[:], in_=mdr[w, :, :])

                adb = ew.tile([128, 128], F16, tag="adb")
                if ESUB >= 2:
                    adT_ps = psM.tile([128, 128], F16, space="PSUM", tag="sm")
                    nc.tensor.transpose(out=adT_ps[:],
                                        in_=adcols[:, w:w + 1].to_broadcast([128, 128]),
                                        identity=idn_t[:])
                    nc.scalar.activation(out=adb[:], in_=adT_ps[:], func=ACT.Copy)
                else:
                    nc.vector.memset(adb[:], 0.0)

                G = gth.tile([128, T, WT], F16, tag="G")
                nc.gpsimd.dma_gather(
                    out_ap=G[:, 0:T0, :], in_ap=tabs[0], idxs_ap=idx_t[:, 0:S0],
                    num_idxs=T0 * 128, num_idxs_reg=T0 * 128, elem_size=WT,
                    single_packet=False)
                nc.gpsimd.dma_gather(
                    out_ap=G[:, T0:T, :], in_ap=tabs[1], idxs_ap=idx_t[:, S0:S],
                    num_idxs=T1T * 128, num_idxs_reg=T1T * 128, elem_size=WT,
                    single_packet=False)

                if ESUB < 3:
                    tpT0 = epi.tile([128, 128], F16, tag="tpT")
                    nc.vector.tensor_copy(out=tpT0[:], in_=G[:, 0, 0:128])
                    nc.sync.dma_start(
                        out=outT_dram[0:128, w * 128:(w + 1) * 128], in_=tpT0[:])
                    return
                oh = ohb.tile([128, T, 128], F16, tag="oh")
                adx = ew.tile([128, T], F32, tag="adx")
                scr = ew.tile([128, 128], F16, tag="scr")
                for t in range(T):
                    nc.vector.tensor_tensor(
                        out=oh[:, t, :], in0=iota_t[:],
                        in1=dr_t[:, t:t + 1].to_broadcast([128, 128]),
                        op=AOP.is_equal)
                    if ESUB >= 32:
                        nc.vector.tensor_tensor(out=scr[:], in0=oh[:, t, :],
                                                in1=adb[:], op=AOP.mult)
                        nc.vector.reduce_sum(out=adx[:, t:t + 1], in_=scr[:],
                                             axis=mybir.AxisListType.X)
                    else:
                        nc.vector.reduce_sum(out=adx[:, t:t + 1], in_=oh[:, t, :],
                                             axis=mybir.AxisListType.X)
                a_sc = ew.tile([128, T], F32, tag="asc")
                if ESUB >= 33:
                    nc.vector.tensor_copy(out=a_sc[:], in_=G[:, :, H])
                else:
                    nc.vector.memset(a_sc[:], 0.0)
                z = ew.tile([128, T], F32, tag="z")
                nc.vector.tensor_tensor(out=z[:], in0=a_sc[:], in1=adx[:], op=AOP.add)
                z2 = ew.tile([128, T], F32, tag="z2")
                nc.vector.tensor_scalar_mul(out=z2[:], in0=z[:], scalar1=NEG)
                nc.vector.tensor_tensor(out=z[:], in0=z[:], in1=z2[:], op=AOP.max)
                p = ew.tile([128, T], F16, tag="p")
                nc.scalar.activation(out=p[:], in_=z[:], func=ACT.Exp)
                if ESUB < 40:
                    tpT0 = epi.tile([128, 128], F16, tag="tpT")
                    nc.vector.tensor_copy(out=tpT0[:, 0:T], in_=p[:])
                    nc.sync.dma_start(
                        out=outT_dram[0:128, w * 128:(w + 1) * 128], in_=tpT0[:])
                    return

                out_ps = psA.tile([128, H1], F32, space="PSUM", tag="acc")
                s_ps = psS.tile([128, 8], F32, space="PSUM", tag="s")
                for t in range(T):
                    nc.vector.tensor_tensor(
                        out=oh[:, t, :], in0=oh[:, t, :],
                        in1=p[:, t:t + 1].to_broadcast([128, 128]), op=AOP.mult)
                    nc.tensor.matmul(out=out_ps[:, 0:H], lhsT=oh[:, t, :],
                                     rhs=G[:, t, 0:H],
                                     start=(t == 0), stop=(t == T - 1))
                    nc.tensor.matmul(out=s_ps[:, 0:1], lhsT=oh[:, t, :],
                                     rhs=ones_t[:],
                                     start=(t == 0), stop=(t == T - 1))
                if ESUB < 50:
                    tpT0 = epi.tile([128, 128], F16, tag="tpT")
                    nc.vector.tensor_copy(out=tpT0[:], in_=out_ps[:, 0:128])
                    nc.vector.tensor_copy(out=tpT0[:, 0:1], in_=s_ps[:, 0:1])
                    nc.sync.dma_start(
                        out=outT_dram[0:128, w * 128:(w + 1) * 128], in_=tpT0[:])
                    return
                s_sb = ew.tile([128, 1], F32, tag="ssb")
                nc.vector.tensor_scalar_add(out=s_sb[:], in0=s_ps[:, 0:1],
                                            scalar1=1e-16)
                rs = ew.tile([128, 1], F32, tag="rs")
                nc.vector.reciprocal(out=rs[:], in_=s_sb[:])
                res_t = epi.tile([128, H1], F32, tag="res")
                nc.sync.dma_start(out=res_t[:, 0:H],
                                  in_=res_dram[w * 128:(w + 1) * 128, :])
                gat = epi.tile([128, H1], F32, tag="gat")
                nc.vector.tensor_tensor(out=gat[:, 0:H], in0=out_ps[:, 0:H],
                                        in1=rs[:, 0:1].to_broadcast([128, H]),
                                        op=AOP.mult)
                nc.vector.tensor_tensor(out=gat[:, 0:H], in0=gat[:, 0:H],
                                        in1=res_t[:, 0:H], op=AOP.add)
                hh = epi.tile([128, H1], F16, tag="hh")
                nc.scalar.activation(out=hh[:, 0:H], in_=gat[:, 0:H],
                                     func=(ACT.Relu if do_relu else ACT.Copy))
                for c in range(H // 128):
                    tp_ps = psM.tile([128, 128], F16, space="PSUM", tag="sm")
                    nc.tensor.transpose(out=tp_ps[:],
                                        in_=hh[:, c * 128:(c + 1) * 128],
                                        identity=idn_t[:])
                    tpT = epi.tile([128, 128], F16, tag="tpT")
                    nc.scalar.activation(out=tpT[:], in_=tp_ps[:], func=ACT.Copy)
                    nc.sync.dma_start(
                        out=outT_dram[c * 128:(c + 1) * 128,
                                      w * 128:(w + 1) * 128],
                        in_=tpT[:])

            # =========== E1: conv1 edge phase ===========
            for w in range(NW if PHASES >= 3 else 0):
                edge_phase(w, H1, W_T1, (t1a[:, :], t1b[:, :]), ad1_t,
                           res1, True, h1T)

            # =========== D2': own h2 / a_s2 / a_d2 / res2 ===========
            for w in range(NW if PHASES >= 4 else 0):
                hts = []
                for k in range(4):
                    ht = xin.tile([128, 128], F16, tag=f"ht{k}")
                    nc.sync.dma_start(
                        out=ht[:],
                        in_=h1T[k * 128:(k + 1) * 128, w * 128:(w + 1) * 128])
                    hts.append(ht)
                h2_ps = psA.tile([128, H1], F32, space="PSUM", tag="acc")
                s2_ps = psS.tile([128, 8], F32, space="PSUM", tag="s")
                for k in range(4):
                    nc.tensor.matmul(out=h2_ps[:, 0:H2], lhsT=hts[k][:],
                                     rhs=w2_t[:, k * H2:(k + 1) * H2],
                                     start=(k == 0), stop=(k == 3))
                    nc.tensor.matmul(out=s2_ps[:, 0:2], lhsT=hts[k][:],
                                     rhs=wv2_t[:, 2 * k:2 * k + 2],
                                     start=(k == 0), stop=(k == 3))
                nc.vector.tensor_copy(out=ad2_t[:, w:w + 1], in_=s2_ps[:, 1:2])
                t2row = hxp.tile([128, W_T2], F16, tag="t2row")
                nc.scalar.activation(out=t2row[:, 0:H2], in_=h2_ps[:, 0:H2],
                                     func=ACT.Copy)
                nc.vector.tensor_copy(out=t2row[:, H2:H2 + 1], in_=s2_ps[:, 0:1])
                if w < 25:
                    nc.sync.dma_start(out=t2inA[w * 128:(w + 1) * 128, :],
                                      in_=t2row[:])
                else:
                    nc.sync.dma_start(out=t2inB[(w - 25) * 128:(w - 24) * 128, :],
                                      in_=t2row[:])
                r2_ps = psA.tile([128, H1], F32, space="PSUM", tag="acc")
                for k in range(4):
                    nc.tensor.matmul(out=r2_ps[:, 0:H2], lhsT=hts[k][:],
                                     rhs=a2_t[:, k * H2:(k + 1) * H2],
                                     start=(k == 0), stop=(k == 3))
                r2_sb = epi.tile([128, H2], F32, tag="r2")
                nc.vector.tensor_tensor(out=r2_sb[:], in0=r2_ps[:, 0:H2],
                                        in1=b2r_t[:], op=AOP.add)
                nc.sync.dma_start(out=res2[w * 128:(w + 1) * 128, :], in_=r2_sb[:])

            # =========== AllGather T2 table (2 banks) ===========
            if PHASES >= 5:
              nc.gpsimd.collective_compute(
                "AllGather", AOP.bypass, replica_groups=rg,
                ins=[t2inA[:, :]], outs=[t2a[:, :, :]])
              nc.gpsimd.collective_compute(
                "AllGather", AOP.bypass, replica_groups=rg,
                ins=[t2inB[:, :]], outs=[t2b[:, :, :]])

            if KDUMP:
                with tc.tile_pool(name="dbg", bufs=1) as dbgp:
                    bt = dbgp.tile([128, W_T2], F16)
                    nc.sync.dma_start(out=bt[:], in_=t2inA[0:128, :])
                    nc.sync.dma_start(out=t2indbg[:, :], in_=bt[:])
                    for bi, srcap in enumerate((t2a[0, 0:128, :], t2a[2, 0:128, :],
                                                t2b[0, 0:128, :])):
                        bt2 = dbgp.tile([128, W_T2], F16, tag="bt2")
                        nc.sync.dma_start(out=bt2[:], in_=srcap)
                        nc.sync.dma_start(
                            out=t2adbg[bi * 128:(bi + 1) * 128, :], in_=bt2[:])

            # =========== E2: conv2 edge phase ===========
            t2a_flat = t2a[:, :, :].rearrange("o r w -> (o r) w")
            t2b_flat = t2b[:, :, :].rearrange("o r w -> (o r) w")
            for w in range(NW if PHASES >= 6 else 0):
                edge_phase(w, H2, W_T2, (t2a_flat, t2b_flat), ad2_t,
                           res2, False, h2T)

            # =========== D3: MLP + classifier + softmax on own rows ===========
            for w in range(NW if PHASES >= 7 else 0):
                cur = []
                for k in range(2):
                    ht = xin.tile([128, 128], F16, tag=f"h2t{k}")
                    nc.sync.dma_start(
                        out=ht[:],
                        in_=h2T[k * 128:(k + 1) * 128, w * 128:(w + 1) * 128])
                    cur.append(ht)
                for l in range(3):
                    nxt = []
                    for ch in range(2):
                        l_ps = psM.tile([128, 128], F32, space="PSUM", tag="sm")
                        for k in range(2):
                            nc.tensor.matmul(
                                out=l_ps[:],
                                lhsT=hw_t[:, (l * 2 + k) * H2 + ch * 128:
                                          (l * 2 + k) * H2 + (ch + 1) * 128],
                                rhs=cur[k][:], start=(k == 0), stop=(k == 1))
                        nx = ew.tile([128, 128], F16, tag=f"nx{ch}")
                        nc.scalar.activation(out=nx[:], in_=l_ps[:], func=ACT.Relu,
                                             bias=hbs_t[:, l * 2 + ch:l * 2 + ch + 1])
                        nxt.append(nx)
                    cur = nxt
                lg_ps = psM.tile([128, 128], F32, space="PSUM", tag="sm")
                for k in range(2):
                    nc.tensor.matmul(out=lg_ps[:, 0:NCLS], lhsT=cur[k][:],
                                     rhs=fcw_t[:, k * NCLS:(k + 1) * NCLS],
                                     start=(k == 0), stop=(k == 1))
                lg = ew.tile([128, NCLS], F32, tag="lg")
                nc.vector.tensor_tensor(out=lg[:], in0=lg_ps[:, 0:NCLS],
                                        in1=fcbr_t[:], op=AOP.add)
                if KDUMP:
                    nc.sync.dma_start(out=lgdbg[w * 128:(w + 1) * 128, :], in_=lg[:])
                mx = ew.tile([128, 1], F32, tag="mx")
                nc.vector.reduce_max(out=mx[:], in_=lg[:], axis=mybir.AxisListType.X)
                nmx = ew.tile([128, 1], F32, tag="nmx")
                nc.vector.tensor_scalar_mul(out=nmx[:], in0=mx[:], scalar1=-1.0)
                ex = ew.tile([128, NCLS], F32, tag="ex")
                nc.scalar.activation(out=ex[:], in_=lg[:], func=ACT.Exp,
                                     bias=nmx[:, 0:1])
                sm = ew.tile([128, 1], F32, tag="smm")
                nc.vector.reduce_sum(out=sm[:], in_=ex[:], axis=mybir.AxisListType.X)
                rsm = ew.tile([128, 1], F32, tag="rsm")
                nc.vector.reciprocal(out=rsm[:], in_=sm[:])
                ob = ew.tile([128, NCLS], F32, tag="ob")
                nc.vector.tensor_tensor(out=ob[:], in0=ex[:],
                                        in1=rsm[:, 0:1].to_broadcast([128, NCLS]),
                                        op=AOP.mult)
                nc.sync.dma_start(out=outp[w * 128:(w + 1) * 128, :], in_=ob[:])

    nc.compile()
    return nc


def _install_trace_shims():
    """Test-only (KTRACE=1): provide the NTFF profile hook this image lacks
    and stub the S3 artifact upload (zero-egress container)."""
    import types, contextlib, ctypes
    import concourse.bass_utils as bu
    bu.upload_artifacts = lambda d: d
    try:
        from antenv.axon_hooks import get_axon_ntff_profile_hook  # noqa
        return
    except ImportError:
        pass
    lib = ctypes.CDLL('/opt/axon/libaxon_pjrt.so')
    if not hasattr(lib, 'axon_start_nrt_profile'):
        return
    lib.axon_start_nrt_profile.argtypes = [ctypes.POINTER(ctypes.c_int64),
                                           ctypes.c_size_t]
    lib.axon_start_nrt_profile.restype = ctypes.c_int64
    lib.axon_stop_nrt_profile.argtypes = [ctypes.c_char_p]
    lib.axon_stop_nrt_profile.restype = ctypes.c_int64

    @contextlib.contextmanager
    def _hook(output_dir, device_ids):
        import jax
        jax.devices()
        if device_ids:
            ids = (ctypes.c_int64 * len(device_ids))(*device_ids)
            rc = lib.axon_start_nrt_profile(ids, len(device_ids))
        else:
            rc = lib.axon_start_nrt_profile(None, 0)
        if rc != 0:
            raise RuntimeError(f"axon_start_nrt_profile rc={rc}")
        try:
            yield
        finally:
            n = lib.axon_stop_nrt_profile(str(output_dir).encode())
            print(f"profile: {n} file(s) written to {output_dir}",
                  file=sys.stderr)

    import antenv
    mod = types.ModuleType('antenv.axon_hooks')
    mod.get_axon_ntff_profile_hook = lambda: _hook
    mod.set_axon_ntff_profile_hook = lambda h: None
    sys.modules['antenv.axon_hooks'] = mod
    antenv.axon_hooks = mod


def _fp16(x):
    return np.ascontiguousarray(np.asarray(x, np.float32).astype(np.float16))


def _preprocess(edge_index, t_bank):
    """Group edges by (dst owner, window, src bank); emit per-core gather
    index / dst_rel arrays. Returns None if t_bank is too small."""
    T0, T1T = t_bank
    T = T0 + T1T
    S0, S1 = T0 * 8, T1T * 8
    src = np.asarray(edge_index[0], np.int64)
    dst = np.asarray(edge_index[1], np.int64)
    owner = dst // OWN
    lrow = dst % OWN
    win = lrow // 128
    drel = (lrow % 128).astype(np.float32)
    so = src // OWN
    sr = src % OWN
    bank = (sr >= QSTART[1]).astype(np.int64)
    srow = so * np.where(bank == 0, QLEN[0], QLEN[1]) + sr - np.where(
        bank == 0, QSTART[0], QSTART[1])

    # sort edges by (owner, window, bank) with a stable counting sort
    key = ((owner * NW + win) * 2 + bank)
    order = np.argsort(key, kind="stable")
    key_s = key[order]
    srow_s = srow[order]
    drel_s = drel[order]
    bounds = np.searchsorted(key_s, np.arange(M * NW * 2 + 1))

    midx = np.zeros((M, NW, 128, S0 + S1), np.int16)
    mdr = np.full((M, NW, 128, T), -1.0, np.float32)
    for c in range(M):
        for w in range(NW):
            for b in range(2):
                k = (c * NW + w) * 2 + b
                lo, hi = bounds[k], bounds[k + 1]
                n = hi - lo
                tmax = (T0 if b == 0 else T1T) * 128
                if n > tmax:
                    return None
                gi = np.zeros(tmax, np.int64)
                gi[:n] = srow_s[lo:hi]
                dr = np.full(tmax, -1.0, np.float32)
                dr[:n] = drel_s[lo:hi]
                # gather idx wrapped: edge i -> [i%16, i//16], tiled to 128 rows
                wrp = gi.reshape(-1, 16).T.astype(np.int16)   # [16, tmax/16]
                co = 0 if b == 0 else S0
                midx[c, w, :, co:co + tmax // 16] = np.tile(wrp, (8, 1))
                # dst_rel: edge i -> [i%128, i//128]
                to = 0 if b == 0 else T0
                mdr[c, w, :, to:to + tmax // 128] = dr.reshape(-1, 128).T
    return midx, mdr


def kernel(**inputs):
    x = np.asarray(inputs["x"], np.float32)
    edge_index = np.asarray(inputs["edge_index"])
    W1 = np.asarray(inputs["W1"], np.float32)
    A1 = np.asarray(inputs["A1"], np.float32)
    W2 = np.asarray(inputs["W2"], np.float32)
    A2 = np.asarray(inputs["A2"], np.float32)
    att_src1 = np.asarray(inputs["att_src1"], np.float32)
    att_dst1 = np.asarray(inputs["att_dst1"], np.float32)
    att_src2 = np.asarray(inputs["att_src2"], np.float32)
    att_dst2 = np.asarray(inputs["att_dst2"], np.float32)
    b1 = np.asarray(inputs["b1"], np.float32) + np.asarray(inputs["b_conv1"], np.float32)
    b2 = np.asarray(inputs["b2"], np.float32) + np.asarray(inputs["b_conv2"], np.float32)
    Hw = [np.asarray(inputs[f"Hw{i}"], np.float32) for i in (1, 2, 3)]
    Hb = [np.asarray(inputs[f"Hb{i}"], np.float32) for i in (1, 2, 3)]
    fcw = np.asarray(inputs["fcw"], np.float32)
    fcb = np.asarray(inputs["fcb"], np.float32)

    # ---- preprocess edges; grow tile budget if this graph needs more ----
    t_bank = T_BANK_DEFAULT
    pre = _preprocess(edge_index, t_bank)
    while pre is None:
        t_bank = (t_bank[0] + 1, t_bank[1] + 1)
        pre = _preprocess(edge_index, t_bank)
    midx, mdr = pre

    key = (t_bank, PHASES, ESUB, KDUMP)
    if key not in _CACHE:
        _CACHE[key] = _build_program(t_bank)
    nc = _CACHE[key]

    # ---- host-side weight packing ----
    xpad = np.zeros((NPAD, D_IN), np.float32)
    xpad[:N] = x
    xT = _fp16(xpad.T)                                   # [256, 50176]
    w1s = _fp16(W1.reshape(2, 128, H1).transpose(1, 0, 2).reshape(128, 2 * H1))
    a1s = _fp16(A1.reshape(2, 128, H1).transpose(1, 0, 2).reshape(128, 2 * H1))
    w2s = _fp16(W2.reshape(4, 128, H2).transpose(1, 0, 2).reshape(128, 4 * H2))
    a2s = _fp16(A2.reshape(4, 128, H2).transpose(1, 0, 2).reshape(128, 4 * H2))
    hws = _fp16(np.concatenate(
        [w.reshape(2, 128, H2).transpose(1, 0, 2).reshape(128, 2 * H2) for w in Hw],
        axis=1))
    fcws = _fp16(fcw.reshape(2, 128, NCLS).transpose(1, 0, 2).reshape(128, 2 * NCLS))
    was1 = W1 @ att_src1
    wad1 = W1 @ att_dst1
    was2 = W2 @ att_src2
    wad2 = W2 @ att_dst2
    wv1 = _fp16(np.stack([was1[:128], was1[128:], wad1[:128], wad1[128:]], axis=1))
    wv2_cols = []
    for k in range(4):
        wv2_cols.append(was2[k * 128:(k + 1) * 128])
        wv2_cols.append(wad2[k * 128:(k + 1) * 128])
    wv2 = _fp16(np.stack(wv2_cols, axis=1))
    b1r = np.ascontiguousarray(np.broadcast_to(b1, (128, H1)), np.float32)
    b2r = np.ascontiguousarray(np.broadcast_to(b2, (128, H2)), np.float32)
    hbs = np.stack([Hb[l][ch * 128:(ch + 1) * 128]
                    for l in range(3) for ch in range(2)], axis=1).astype(np.float32)
    fcbr = np.ascontiguousarray(np.broadcast_to(fcb, (128, NCLS)), np.float32)
    iota16 = np.ascontiguousarray(
        np.broadcast_to(np.arange(128, dtype=np.float16), (128, 128)))
    idn16 = np.eye(128, dtype=np.float16)

    in_maps = []
    for c in range(M):
        in_maps.append({
            "xTg": xT,
            "xTo": np.ascontiguousarray(xT[:, c * OWN:(c + 1) * OWN]),
            "w1s": w1s, "a1s": a1s, "w2s": w2s, "a2s": a2s,
            "hws": hws, "fcws": fcws, "wv1": wv1, "wv2": wv2,
            "b1r": b1r, "b2r": b2r, "hbs": hbs, "fcbr": fcbr,
            "iota16": iota16, "idn16": idn16,
            "midx": midx[c], "mdr": mdr[c],
        })

    trace = os.environ.get("KTRACE", "0") == "1"
    tkw = {}
    if trace:
        _install_trace_shims()
        tdir = os.environ.get("KTRACE_DIR")
        if tdir:
            os.makedirs(tdir, exist_ok=True)
        tkw = dict(trace=True, tmpdir=tdir,
                   trace_cores=[int(c) for c in
                                os.environ.get("KTRACE_CORES", "0").split(",")])
    res = run_bass_kernel_spmd(nc, in_maps, core_ids=list(range(M)), **tkw)
    if trace:
        sys.modules[__name__].last_exec_time_ns = res.exec_time_ns
        sys.modules[__name__].last_trace_path = (
            res.instructions_and_trace[1] if res.instructions_and_trace else None)
    if KDUMP:
        kernel.last_results = res.results
    out = np.concatenate([res.results[c]["outp"] for c in range(M)], axis=0)
    return np.ascontiguousarray(out[:N], np.float32)

